# revision 5
# baseline (speedup 1.0000x reference)
"""DCGRU cell Trainium2 kernel.

Math (per batch i):
  xs = [input, state]                                  [N, 66]
  aggr[j] = S[j] @ xs          (J=4 supports)          [N, 66]
  r = sigmoid(sum_j aggr[j] @ Wr[j] + br)              [N, 64]
  u = sigmoid(sum_j aggr[j] @ Wu[j] + bu)
  xc = [input, r*state]
  c = tanh(sum_j (S[j] @ xc) @ Wc[j] + bc)
  out = u*state + (1-u)*c

Sharding: data-parallel over batch, 8 batches per core on 8 cores.
supports/weights replicated. No collectives.

Device kernel layout choices (per core, Bc=8):
  - Host pre-transposes supports: ST[j] = S[j].T (so [m, k], m = contraction
    dim) -> stationary matmul operands are contiguous row-blocks.
  - XS packed [m=2048, (i=8, f=66)]: moving operand, SBUF resident.
  - Big matmul (per j, k-block kb, m-block mb):
      psum[kb][:, c] += ST[j][mb, kb*128:+128].T @ XS[mb][:, c]
    i.e. aggr[j].T is never formed; aggr[j] comes out [k, (i,f)].
  - PE-transpose [128, 66] slices -> aggT[i][j] [66, kchunk], then the
    W-projection (contract 66, accumulate over j in PSUM), bias+activation
    via ScalarE, output ru.T [128 = (r|u), k] per batch.
  - Phase 2 identical with xc; c.T overwrites the dead r.T storage.
  - GRU combine on DVE in [64, N] layout; host undoes the final transpose.
"""

import sys

if '/opt/trn_rl_repo' not in sys.path:
    sys.path.insert(0, '/opt/trn_rl_repo')

import numpy as np

B, N, IN, OUT, J = 64, 2048, 2, 64, 4
NCORES = 8
BC = B // NCORES            # 8 batches per core
F = IN + OUT                # 66
CB = BC * F                 # 528 moving columns, phase big-matmul
P = 128
HALF = CB // 2              # 264 (psum bank split)
NMB = N // P                # 16 m blocks
NKB = N // P                # 16 k blocks
KBG = 2                     # k blocks per psum group
NG = NKB // KBG             # 8 groups
MBQ = 4                     # m blocks per ST dma

_CACHE = {}


def _build_module():
    import concourse.tile as tile
    import concourse.mybir as mybir
    from concourse import bacc
    from concourse.masks import make_identity

    f32 = mybir.dt.float32
    f32r = mybir.dt.float32r
    AF = mybir.ActivationFunctionType

    nc = bacc.Bacc("TRN2", target_bir_lowering=False, debug=False,
                   num_devices=1)

    st_d = nc.dram_tensor("st", [J, N, N], f32r, kind="ExternalInput").ap()
    xs_d = nc.dram_tensor("xs", [N, CB], f32r, kind="ExternalInput").ap()
    xin_d = nc.dram_tensor("xin", [N, BC * IN], f32, kind="ExternalInput").ap()
    stT_d = nc.dram_tensor("stT", [BC, OUT, N], f32, kind="ExternalInput").ap()
    wru_d = nc.dram_tensor("wru", [J, F, 2 * OUT], f32r, kind="ExternalInput").ap()
    wc_d = nc.dram_tensor("wc", [J, F, OUT], f32r, kind="ExternalInput").ap()
    bru_d = nc.dram_tensor("bru", [2 * OUT, 1], f32, kind="ExternalInput").ap()
    bc_d = nc.dram_tensor("bc", [OUT, 1], f32, kind="ExternalInput").ap()
    outT_d = nc.dram_tensor("outT", [BC, OUT, N], f32, kind="ExternalOutput").ap()

    with tile.TileContext(nc) as tc:
        with tc.tile_pool(name="const", bufs=1) as const_pool, \
             tc.tile_pool(name="xs", bufs=18) as xs_pool, \
             tc.tile_pool(name="xin", bufs=16) as xin_pool, \
             tc.tile_pool(name="ruT", bufs=BC) as ruT_pool, \
             tc.tile_pool(name="stT", bufs=2) as stT_pool:

            ident = const_pool.tile([P, P], f32, tag="ident")
            make_identity(nc, ident[:])

            wru_t = []
            wc_t = []
            for j in range(J):
                w1 = const_pool.tile([F, 2 * OUT], f32r, tag=f"wru{j}")
                nc.sync.dma_start(w1[:], wru_d[j])
                wru_t.append(w1)
                w2 = const_pool.tile([F, OUT], f32r, tag=f"wc{j}")
                nc.sync.dma_start(w2[:], wc_d[j])
                wc_t.append(w2)
            bru_t = const_pool.tile([2 * OUT, 1], f32, tag="bru")
            nc.sync.dma_start(bru_t[:], bru_d[:])
            bc_t = const_pool.tile([OUT, 1], f32, tag="bc")
            nc.sync.dma_start(bc_t[:], bc_d[:])

            xs_tiles = []
            for mb in range(NMB):
                t = xs_pool.tile([P, CB], f32r, tag="xs")
                nc.sync.dma_start(t[:], xs_d[mb * P:(mb + 1) * P, :])
                xs_tiles.append(t)
            xin_tiles = []
            for mb in range(NMB):
                t = xin_pool.tile([P, BC * IN], f32, tag="xin")
                nc.sync.dma_start(t[:], xin_d[mb * P:(mb + 1) * P, :])
                xin_tiles.append(t)

            ruT_tiles = [ruT_pool.tile([P, N], f32, tag="ruT", name=f"ruT{i}")
                         for i in range(BC)]

            def big_phase(x_tiles, w_tiles, out_rows, bias_t, act_fn,
                          out_slice_fn):
                """One graph-conv pass + projection + activation.

                out_slice_fn(i, k0, width) -> SBUF AP [out_rows, width]
                receiving act(proj + bias) for batch i, k columns
                [k0, k0+width).
                """
                with tc.tile_pool(name="stst", bufs=2) as st_pool, \
                     tc.tile_pool(name="agg", bufs=16) as agg_pool, \
                     tc.tile_pool(name="aggT", bufs=8) as aggT_pool, \
                     tc.tile_pool(name="aggps", bufs=4, space="PSUM") as agg_ps_pool, \
                     tc.tile_pool(name="tpps", bufs=2, space="PSUM") as tp_ps_pool, \
                     tc.tile_pool(name="projps", bufs=2, space="PSUM") as proj_ps_pool:
                    for g in range(NG):
                        k0 = g * KBG * P        # 256-aligned k offset
                        agg_sb = {}
                        for j in range(J):
                            st_ts = []
                            for mq in range(NMB // MBQ):
                                st_t = st_pool.tile([P, MBQ, KBG * P], f32r,
                                                    tag="st")
                                src = st_d[j, mq * MBQ * P:(mq + 1) * MBQ * P,
                                           k0:k0 + KBG * P]
                                src = src.rearrange("(g p) k -> p g k", p=P)
                                nc.sync.dma_start(st_t[:], src)
                                st_ts.append(st_t)
                            ps = [[agg_ps_pool.tile([P, HALF], f32, tag="aggps", name=f"aggps{kb}_{h}")
                                   for h in range(2)] for kb in range(KBG)]
                            for mb in range(NMB):
                                mq, ml = divmod(mb, MBQ)
                                for kb in range(KBG):
                                    lhsT = st_ts[mq][:, ml, kb * P:(kb + 1) * P]
                                    for h in range(2):
                                        nc.tensor.matmul(
                                            ps[kb][h][:],
                                            lhsT,
                                            x_tiles[mb][:, h * HALF:(h + 1) * HALF],
                                            start=(mb == 0),
                                            stop=(mb == NMB - 1),
                                        )
                            for kb in range(KBG):
                                t = agg_pool.tile([P, CB], f32, tag="agg")
                                for h in range(2):
                                    nc.vector.tensor_copy(
                                        t[:, h * HALF:(h + 1) * HALF],
                                        ps[kb][h][:])
                                agg_sb[(j, kb)] = t

                        for i in range(BC):
                            aggT = []
                            for j in range(J):
                                at = aggT_pool.tile([F, KBG * P], f32r,
                                                    tag="aggT")
                                for kb in range(KBG):
                                    tp = tp_ps_pool.tile([F, P], f32, tag="tp")
                                    nc.tensor.transpose(
                                        tp[:],
                                        agg_sb[(j, kb)][:, i * F:(i + 1) * F],
                                        ident[:])
                                    nc.vector.tensor_copy(
                                        at[:, kb * P:(kb + 1) * P], tp[:])
                                aggT.append(at)
                            pp = proj_ps_pool.tile([out_rows, KBG * P], f32,
                                                   tag="proj")
                            for j in range(J):
                                nc.tensor.matmul(
                                    pp[:],
                                    w_tiles[j][:],
                                    aggT[j][:],
                                    start=(j == 0),
                                    stop=(j == J - 1),
                                )
                            nc.scalar.activation(
                                out_slice_fn(i, k0, KBG * P), pp[:], act_fn,
                                bias=bias_t[:, 0:1])

            # ---- phase 1: r|u = sigmoid(graph_conv(xs, Wr|Wu)) ----
            big_phase(
                xs_tiles, wru_t, 2 * OUT, bru_t, AF.Sigmoid,
                lambda i, k0, w: ruT_tiles[i][:, k0:k0 + w])

            # ---- boundary: xc = [input, r*state] in [m, (i,f)] layout ----
            xc_tiles = [xs_pool.tile([P, CB], f32r, tag="xs", name=f"xc{mb}")
                        for mb in range(NMB)]
            with tc.tile_pool(name="rstp", bufs=2, space="PSUM") as rstp_pool, \
                 tc.tile_pool(name="rsT", bufs=2) as rsT_pool:
                for mb in range(NMB):
                    dst = xc_tiles[mb][:].rearrange("p (i f) -> p i f", f=F)
                    src = xin_tiles[mb][:].rearrange("p (i f) -> p i f", f=IN)
                    nc.vector.tensor_copy(dst[:, :, 0:IN], src)
                for i in range(BC):
                    stt = stT_pool.tile([OUT, N], f32, tag="stT")
                    nc.sync.dma_start(stt[:], stT_d[i])
                    rst = rsT_pool.tile([OUT, N], f32, tag="rsT")
                    nc.vector.tensor_mul(rst[:], ruT_tiles[i][0:OUT, :],
                                         stt[:])
                    for mb in range(NMB):
                        tp = rstp_pool.tile([P, OUT], f32, tag="rstp")
                        nc.tensor.transpose(tp[:], rst[:, mb * P:(mb + 1) * P],
                                            ident[0:OUT, 0:OUT])
                        nc.vector.tensor_copy(
                            xc_tiles[mb][:, i * F + IN:(i + 1) * F], tp[:])

            # ---- phase 2: c.T = tanh(proj) overwrites dead r.T rows ----
            big_phase(
                xc_tiles, wc_t, OUT, bc_t, AF.Tanh,
                lambda i, k0, w: ruT_tiles[i][0:OUT, k0:k0 + w])

            # ---- GRU combine: out = c + u*(state - c) ----
            with tc.tile_pool(name="tmp", bufs=3) as tmp_pool:
                for i in range(BC):
                    stt = stT_pool.tile([OUT, N], f32, tag="stT")
                    nc.sync.dma_start(stt[:], stT_d[i])
                    u0 = tmp_pool.tile([OUT, N], f32, tag="tmp")
                    # partition-base shift (64 -> 0) needs a DMA, not DVE
                    nc.sync.dma_start(u0[:], ruT_tiles[i][OUT:2 * OUT, :])
                    t1 = tmp_pool.tile([OUT, N], f32, tag="tmp")
                    nc.vector.tensor_sub(t1[:], stt[:], ruT_tiles[i][0:OUT, :])
                    t2 = tmp_pool.tile([OUT, N], f32, tag="tmp")
                    nc.vector.tensor_mul(t2[:], u0[:], t1[:])
                    t3 = tmp_pool.tile([OUT, N], f32, tag="tmp")
                    nc.vector.tensor_add(t3[:], ruT_tiles[i][0:OUT, :], t2[:])
                    nc.sync.dma_start(outT_d[i], t3[:])

    nc.compile()
    return nc


def _get_module():
    if "nc" not in _CACHE:
        _CACHE["nc"] = _build_module()
    return _CACHE["nc"]


def kernel(input, state, supports, Wr, br, Wu, bu, Wc, bc):
    input = np.asarray(input, np.float32)
    state = np.asarray(state, np.float32)
    supports = np.asarray(supports, np.float32)
    Wr = np.asarray(Wr, np.float32)
    br = np.asarray(br, np.float32)
    Wu = np.asarray(Wu, np.float32)
    bu = np.asarray(bu, np.float32)
    Wc = np.asarray(Wc, np.float32)
    bc = np.asarray(bc, np.float32)

    from concourse.bass_utils import run_bass_kernel_spmd

    nc = _get_module()

    st_host = np.ascontiguousarray(supports.transpose(0, 2, 1))
    wru = np.ascontiguousarray(np.concatenate([Wr, Wu], axis=2))
    bru = np.concatenate([br, bu]).reshape(2 * OUT, 1).astype(np.float32)
    bcc = bc.reshape(OUT, 1).astype(np.float32)
    xs_full = np.concatenate([input, state], axis=2)  # [B, N, F]

    in_maps = []
    for c in range(NCORES):
        sl = slice(c * BC, (c + 1) * BC)
        xs_c = np.ascontiguousarray(
            xs_full[sl].transpose(1, 0, 2).reshape(N, CB))
        xin_c = np.ascontiguousarray(
            input[sl].transpose(1, 0, 2).reshape(N, BC * IN))
        stT_c = np.ascontiguousarray(state[sl].transpose(0, 2, 1))
        in_maps.append({
            "st": st_host,
            "xs": xs_c,
            "xin": xin_c,
            "stT": stT_c,
            "wru": wru,
            "wc": np.ascontiguousarray(Wc),
            "bru": bru,
            "bc": bcc,
        })

    import time
    t0 = time.monotonic()
    res = run_bass_kernel_spmd(nc, in_maps, core_ids=list(range(NCORES)))
    _CACHE["last_wall_s"] = time.monotonic() - t0

    out = np.empty((B, N, OUT), np.float32)
    for c in range(NCORES):
        outT = res.results[c]["outT"]           # [BC, OUT, N]
        out[c * BC:(c + 1) * BC] = outT.transpose(0, 2, 1)
    return out


# revision 8
# speedup vs baseline: 1138.6765x; 1138.6765x over previous
"""DCGRU cell Trainium2 kernel.

Math (per batch i):
  xs = [input, state]                                  [N, 66]
  aggr[j] = S[j] @ xs          (J=4 supports)          [N, 66]
  r = sigmoid(sum_j aggr[j] @ Wr[j] + br)              [N, 64]
  u = sigmoid(sum_j aggr[j] @ Wu[j] + bu)
  xc = [input, r*state]
  c = tanh(sum_j (S[j] @ xc) @ Wc[j] + bc)
  out = u*state + (1-u)*c

Sharding: data-parallel over batch, 8 batches per core on 8 cores.
supports/weights replicated. No collectives.

Device kernel layout choices (per core, Bc=8):
  - Host pre-transposes supports: ST[j] = S[j].T (so [m, k], m = contraction
    dim) -> stationary matmul operands are contiguous row-blocks.
  - XS packed [m=2048, (i=8, f=66)]: moving operand, SBUF resident.
  - Big matmul (per j, k-block kb, m-block mb):
      psum[kb][:, c] += ST[j][mb, kb*128:+128].T @ XS[mb][:, c]
    i.e. aggr[j].T is never formed; aggr[j] comes out [k, (i,f)].
  - PE-transpose [128, 66] slices -> aggT[i][j] [66, kchunk], then the
    W-projection (contract 66, accumulate over j in PSUM), bias+activation
    via ScalarE, output ru.T [128 = (r|u), k] per batch.
  - Phase 2 identical with xc; c.T overwrites the dead r.T storage.
  - GRU combine on DVE in [64, N] layout; host undoes the final transpose.
"""

import sys

if '/opt/trn_rl_repo' not in sys.path:
    sys.path.insert(0, '/opt/trn_rl_repo')

import numpy as np

B, N, IN, OUT, J = 64, 2048, 2, 64, 4
NCORES = 8
BC = B // NCORES            # 8 batches per core
F = IN + OUT                # 66
CB = BC * F                 # 528 moving columns, phase big-matmul
P = 128
HALF = CB // 2              # 264 (psum bank split)
NMB = N // P                # 16 m blocks
NKB = N // P                # 16 k blocks
KBG = 2                     # k blocks per psum group
NG = NKB // KBG             # 8 groups
MBQ = 4                     # m blocks per ST dma

MM16 = True                 # fp16 big-matmul operands (vs float32r)

_CACHE = {}


def _build_module():
    import concourse.tile as tile
    import concourse.mybir as mybir
    from concourse import bacc
    from concourse.masks import make_identity

    f32 = mybir.dt.float32
    f32r = mybir.dt.float32r
    mmdt = mybir.dt.float16 if MM16 else f32r
    AF = mybir.ActivationFunctionType

    nc = bacc.Bacc("TRN2", target_bir_lowering=False, debug=False,
                   num_devices=1)

    st_d = nc.dram_tensor("st", [J, N, N], mmdt, kind="ExternalInput").ap()
    xs_d = nc.dram_tensor("xs", [N, CB], mmdt, kind="ExternalInput").ap()
    xin_d = nc.dram_tensor("xin", [N, BC * IN], f32, kind="ExternalInput").ap()
    stT_d = nc.dram_tensor("stT", [BC, OUT, N], f32, kind="ExternalInput").ap()
    wru_d = nc.dram_tensor("wru", [J, F, 2 * OUT], f32r, kind="ExternalInput").ap()
    wc_d = nc.dram_tensor("wc", [J, F, OUT], f32r, kind="ExternalInput").ap()
    bru_d = nc.dram_tensor("bru", [2 * OUT, 1], f32, kind="ExternalInput").ap()
    bc_d = nc.dram_tensor("bc", [OUT, 1], f32, kind="ExternalInput").ap()
    outT_d = nc.dram_tensor("outT", [BC, OUT, N], f32, kind="ExternalOutput").ap()

    with tile.TileContext(nc) as tc:
        with tc.tile_pool(name="const", bufs=1) as const_pool, \
             tc.tile_pool(name="xs", bufs=18) as xs_pool, \
             tc.tile_pool(name="xin", bufs=16) as xin_pool, \
             tc.tile_pool(name="ruT", bufs=BC) as ruT_pool, \
             tc.tile_pool(name="stT", bufs=2) as stT_pool:

            ident = const_pool.tile([P, P], f32, tag="ident")
            make_identity(nc, ident[:])

            wru_t = []
            wc_t = []
            for j in range(J):
                w1 = const_pool.tile([F, 2 * OUT], f32r, tag=f"wru{j}")
                nc.sync.dma_start(w1[:], wru_d[j])
                wru_t.append(w1)
                w2 = const_pool.tile([F, OUT], f32r, tag=f"wc{j}")
                nc.sync.dma_start(w2[:], wc_d[j])
                wc_t.append(w2)
            bru_t = const_pool.tile([2 * OUT, 1], f32, tag="bru")
            nc.sync.dma_start(bru_t[:], bru_d[:])
            bc_t = const_pool.tile([OUT, 1], f32, tag="bc")
            nc.sync.dma_start(bc_t[:], bc_d[:])

            xs_tiles = []
            for mb in range(NMB):
                t = xs_pool.tile([P, CB], mmdt, tag="xs")
                nc.sync.dma_start(t[:], xs_d[mb * P:(mb + 1) * P, :])
                xs_tiles.append(t)
            xin_tiles = []
            for mb in range(NMB):
                t = xin_pool.tile([P, BC * IN], f32, tag="xin")
                nc.sync.dma_start(t[:], xin_d[mb * P:(mb + 1) * P, :])
                xin_tiles.append(t)

            ruT_tiles = [ruT_pool.tile([P, N], f32, tag="ruT", name=f"ruT{i}")
                         for i in range(BC)]

            def big_phase(x_tiles, w_tiles, out_rows, bias_t, act_fn,
                          out_slice_fn):
                """One graph-conv pass + projection + activation.

                out_slice_fn(i, k0, width) -> SBUF AP [out_rows, width]
                receiving act(proj + bias) for batch i, k columns
                [k0, k0+width).
                """
                with tc.tile_pool(name="stst", bufs=3) as st_pool, \
                     tc.tile_pool(name="agg", bufs=16) as agg_pool, \
                     tc.tile_pool(name="aggT", bufs=8) as aggT_pool, \
                     tc.tile_pool(name="aggps", bufs=4, space="PSUM") as agg_ps_pool, \
                     tc.tile_pool(name="tpps", bufs=2, space="PSUM") as tp_ps_pool, \
                     tc.tile_pool(name="projps", bufs=2, space="PSUM") as proj_ps_pool:
                    for g in range(NG):
                        k0 = g * KBG * P        # 256-aligned k offset
                        agg_sb = {}
                        for j in range(J):
                            st_ts = []
                            for mq in range(NMB // MBQ):
                                st_t = st_pool.tile([P, MBQ, KBG * P], mmdt,
                                                    tag="st")
                                src = st_d[j, mq * MBQ * P:(mq + 1) * MBQ * P,
                                           k0:k0 + KBG * P]
                                src = src.rearrange("(g p) k -> p g k", p=P)
                                nc.sync.dma_start(st_t[:], src)
                                st_ts.append(st_t)
                            ps = [[agg_ps_pool.tile([P, HALF], f32, tag="aggps", name=f"aggps{kb}_{h}")
                                   for h in range(2)] for kb in range(KBG)]
                            for mb in range(NMB):
                                mq, ml = divmod(mb, MBQ)
                                for kb in range(KBG):
                                    lhsT = st_ts[mq][:, ml, kb * P:(kb + 1) * P]
                                    for h in range(2):
                                        nc.tensor.matmul(
                                            ps[kb][h][:],
                                            lhsT,
                                            x_tiles[mb][:, h * HALF:(h + 1) * HALF],
                                            start=(mb == 0),
                                            stop=(mb == NMB - 1),
                                        )
                            for kb in range(KBG):
                                t = agg_pool.tile([P, CB], f32, tag="agg")
                                for h in range(2):
                                    if (kb + h) % 2 == 0:
                                        nc.vector.tensor_copy(
                                            t[:, h * HALF:(h + 1) * HALF],
                                            ps[kb][h][:])
                                    else:
                                        nc.scalar.copy(
                                            t[:, h * HALF:(h + 1) * HALF],
                                            ps[kb][h][:])
                                agg_sb[(j, kb)] = t

                        for i in range(BC):
                            aggT = []
                            for j in range(J):
                                at = aggT_pool.tile([F, KBG * P], f32r,
                                                    tag="aggT",
                                                    name=f"aggT{i}_{j}")
                                for kb in range(KBG):
                                    tp = tp_ps_pool.tile([F, P], f32, tag="tp")
                                    nc.tensor.transpose(
                                        tp[:],
                                        agg_sb[(j, kb)][:, i * F:(i + 1) * F],
                                        ident[:])
                                    if (i + j + kb) % 2 == 0:
                                        nc.vector.tensor_copy(
                                            at[:, kb * P:(kb + 1) * P], tp[:])
                                    else:
                                        nc.scalar.copy(
                                            at[:, kb * P:(kb + 1) * P], tp[:])
                                aggT.append(at)
                            pp = proj_ps_pool.tile([out_rows, KBG * P], f32,
                                                   tag="proj")
                            for j in range(J):
                                nc.tensor.matmul(
                                    pp[:],
                                    w_tiles[j][:],
                                    aggT[j][:],
                                    start=(j == 0),
                                    stop=(j == J - 1),
                                )
                            nc.scalar.activation(
                                out_slice_fn(i, k0, KBG * P), pp[:], act_fn,
                                bias=bias_t[:, 0:1])

            # ---- phase 1: r|u = sigmoid(graph_conv(xs, Wr|Wu)) ----
            big_phase(
                xs_tiles, wru_t, 2 * OUT, bru_t, AF.Sigmoid,
                lambda i, k0, w: ruT_tiles[i][:, k0:k0 + w])

            # ---- boundary: xc = [input, r*state] in [m, (i,f)] layout ----
            xc_tiles = [xs_pool.tile([P, CB], mmdt, tag="xs", name=f"xc{mb}")
                        for mb in range(NMB)]
            with tc.tile_pool(name="rstp", bufs=2, space="PSUM") as rstp_pool, \
                 tc.tile_pool(name="rsT", bufs=2) as rsT_pool:
                for mb in range(NMB):
                    dst = xc_tiles[mb][:].rearrange("p (i f) -> p i f", f=F)
                    src = xin_tiles[mb][:].rearrange("p (i f) -> p i f", f=IN)
                    nc.vector.tensor_copy(dst[:, :, 0:IN], src)
                for i in range(BC):
                    stt = stT_pool.tile([OUT, N], f32, tag="stT")
                    nc.sync.dma_start(stt[:], stT_d[i])
                    rst = rsT_pool.tile([OUT, N], f32, tag="rsT")
                    nc.vector.tensor_mul(rst[:], ruT_tiles[i][0:OUT, :],
                                         stt[:])
                    for mb in range(NMB):
                        tp = rstp_pool.tile([P, OUT], f32, tag="rstp")
                        nc.tensor.transpose(tp[:], rst[:, mb * P:(mb + 1) * P],
                                            ident[0:OUT, 0:OUT])
                        nc.vector.tensor_copy(
                            xc_tiles[mb][:, i * F + IN:(i + 1) * F], tp[:])

            # ---- phase 2: c.T = tanh(proj) overwrites dead r.T rows ----
            big_phase(
                xc_tiles, wc_t, OUT, bc_t, AF.Tanh,
                lambda i, k0, w: ruT_tiles[i][0:OUT, k0:k0 + w])

            # ---- GRU combine: out = c + u*(state - c) ----
            with tc.tile_pool(name="tmp", bufs=3) as tmp_pool:
                for i in range(BC):
                    stt = stT_pool.tile([OUT, N], f32, tag="stT")
                    nc.sync.dma_start(stt[:], stT_d[i])
                    u0 = tmp_pool.tile([OUT, N], f32, tag="tmp")
                    # partition-base shift (64 -> 0) needs a DMA, not DVE
                    nc.sync.dma_start(u0[:], ruT_tiles[i][OUT:2 * OUT, :])
                    t1 = tmp_pool.tile([OUT, N], f32, tag="tmp")
                    nc.vector.tensor_sub(t1[:], stt[:], ruT_tiles[i][0:OUT, :])
                    t2 = tmp_pool.tile([OUT, N], f32, tag="tmp")
                    nc.vector.tensor_mul(t2[:], u0[:], t1[:])
                    t3 = tmp_pool.tile([OUT, N], f32, tag="tmp")
                    nc.vector.tensor_add(t3[:], ruT_tiles[i][0:OUT, :], t2[:])
                    nc.sync.dma_start(outT_d[i], t3[:])

    nc.compile()
    return nc


def _get_module():
    if "nc" not in _CACHE:
        _CACHE["nc"] = _build_module()
    return _CACHE["nc"]


def kernel(input, state, supports, Wr, br, Wu, bu, Wc, bc):
    input = np.asarray(input, np.float32)
    state = np.asarray(state, np.float32)
    supports = np.asarray(supports, np.float32)
    Wr = np.asarray(Wr, np.float32)
    br = np.asarray(br, np.float32)
    Wu = np.asarray(Wu, np.float32)
    bu = np.asarray(bu, np.float32)
    Wc = np.asarray(Wc, np.float32)
    bc = np.asarray(bc, np.float32)

    from concourse.bass_utils import run_bass_kernel_spmd

    nc = _get_module()

    mmnp = np.float16 if MM16 else np.float32
    st_host = np.ascontiguousarray(supports.transpose(0, 2, 1).astype(mmnp))
    wru = np.ascontiguousarray(np.concatenate([Wr, Wu], axis=2))
    bru = np.concatenate([br, bu]).reshape(2 * OUT, 1).astype(np.float32)
    bcc = bc.reshape(OUT, 1).astype(np.float32)
    xs_full = np.concatenate([input, state], axis=2)  # [B, N, F]

    in_maps = []
    for c in range(NCORES):
        sl = slice(c * BC, (c + 1) * BC)
        xs_c = np.ascontiguousarray(
            xs_full[sl].transpose(1, 0, 2).reshape(N, CB).astype(mmnp))
        xin_c = np.ascontiguousarray(
            input[sl].transpose(1, 0, 2).reshape(N, BC * IN))
        stT_c = np.ascontiguousarray(state[sl].transpose(0, 2, 1))
        in_maps.append({
            "st": st_host,
            "xs": xs_c,
            "xin": xin_c,
            "stT": stT_c,
            "wru": wru,
            "wc": np.ascontiguousarray(Wc),
            "bru": bru,
            "bc": bcc,
        })

    import time
    t0 = time.monotonic()
    res = run_bass_kernel_spmd(nc, in_maps, core_ids=list(range(NCORES)))
    _CACHE["last_wall_s"] = time.monotonic() - t0

    out = np.empty((B, N, OUT), np.float32)
    for c in range(NCORES):
        outT = res.results[c]["outT"]           # [BC, OUT, N]
        out[c * BC:(c + 1) * BC] = outT.transpose(0, 2, 1)
    return out


# revision 10
# speedup vs baseline: 1199.6158x; 1.0535x over previous
"""DCGRU cell Trainium2 kernel.

Math (per batch i):
  xs = [input, state]                                  [N, 66]
  aggr[j] = S[j] @ xs          (J=4 supports)          [N, 66]
  r = sigmoid(sum_j aggr[j] @ Wr[j] + br)              [N, 64]
  u = sigmoid(sum_j aggr[j] @ Wu[j] + bu)
  xc = [input, r*state]
  c = tanh(sum_j (S[j] @ xc) @ Wc[j] + bc)
  out = u*state + (1-u)*c

Sharding: data-parallel over batch, 8 batches per core on 8 cores.
supports/weights replicated. No collectives.

Device kernel layout (per core, Bc=8):
  - Host pre-transposes supports: ST[j] = S[j].T ([m, k], m = contraction),
    cast fp16 -> stationary matmul operands are contiguous row-block slices.
  - XS packed [m=2048, (i=8, f=66)] fp16: moving operand, SBUF resident.
  - Big matmul accumulates aggr[j] = [k, (i,f)] in PSUM over 16 m-blocks;
    528-col batches split 264+264 across two banks; kb/h-major mb-minor
    order so each bank's drain overlaps the next bank's matmuls.
  - PE-transpose [128, 66] slices -> aggT[i][j] [66, 256], W-projection
    (contract 66, fp32r, accumulate over j in PSUM), bias+activation on
    ScalarE -> ru.T [128 = (r|u), k] per batch.
  - Phase 2 identical with xc; c.T overwrites the dead r.T rows.
  - GRU combine on DVE in [64, N] layout; host undoes the final transpose.
"""

import sys

if '/opt/trn_rl_repo' not in sys.path:
    sys.path.insert(0, '/opt/trn_rl_repo')

import numpy as np

B, N, IN, OUT, J = 64, 2048, 2, 64, 4
NCORES = 8
BC = B // NCORES            # 8 batches per core
F = IN + OUT                # 66
CB = BC * F                 # 528 moving columns
P = 128
HALF = CB // 2              # 264 (psum bank split)
NMB = N // P                # 16 m blocks
NKB = N // P                # 16 k blocks
KBG = 2                     # k blocks per psum group
NG = NKB // KBG             # 8 groups
MBQ = 4                     # m blocks per ST dma

MM16 = True                 # fp16 big-matmul operands (vs float32r)

_CACHE = {}


def _build_module():
    import concourse.tile as tile
    import concourse.mybir as mybir
    from concourse import bacc
    from concourse.masks import make_identity

    f32 = mybir.dt.float32
    f32r = mybir.dt.float32r
    mmdt = mybir.dt.float16 if MM16 else f32r
    AF = mybir.ActivationFunctionType

    nc = bacc.Bacc("TRN2", target_bir_lowering=False, debug=False,
                   num_devices=1)

    st_d = nc.dram_tensor("st", [J, N, N], mmdt, kind="ExternalInput").ap()
    xs_d = nc.dram_tensor("xs", [N, CB], mmdt, kind="ExternalInput").ap()
    xin_d = nc.dram_tensor("xin", [N, BC * IN], f32, kind="ExternalInput").ap()
    stT_d = nc.dram_tensor("stT", [BC, OUT, N], f32, kind="ExternalInput").ap()
    wru_d = nc.dram_tensor("wru", [J, F, 2 * OUT], f32r, kind="ExternalInput").ap()
    wc_d = nc.dram_tensor("wc", [J, F, OUT], f32r, kind="ExternalInput").ap()
    bru_d = nc.dram_tensor("bru", [2 * OUT, 1], f32, kind="ExternalInput").ap()
    bc_d = nc.dram_tensor("bc", [OUT, 1], f32, kind="ExternalInput").ap()
    outT_d = nc.dram_tensor("outT", [BC, OUT, N], f32, kind="ExternalOutput").ap()

    with tile.TileContext(nc) as tc:
        with tc.tile_pool(name="const", bufs=1) as const_pool, \
             tc.tile_pool(name="xs", bufs=18) as xs_pool, \
             tc.tile_pool(name="xin", bufs=16) as xin_pool, \
             tc.tile_pool(name="ruT", bufs=BC) as ruT_pool, \
             tc.tile_pool(name="stT", bufs=2) as stT_pool:

            ident = const_pool.tile([P, P], f32, tag="ident")
            make_identity(nc, ident[:])

            wru_t = []
            wc_t = []
            for j in range(J):
                w1 = const_pool.tile([F, 2 * OUT], f32r, tag=f"wru{j}")
                nc.sync.dma_start(w1[:], wru_d[j])
                wru_t.append(w1)
                w2 = const_pool.tile([F, OUT], f32r, tag=f"wc{j}")
                nc.sync.dma_start(w2[:], wc_d[j])
                wc_t.append(w2)
            bru_t = const_pool.tile([2 * OUT, 1], f32, tag="bru")
            nc.sync.dma_start(bru_t[:], bru_d[:])
            bc_t = const_pool.tile([OUT, 1], f32, tag="bc")
            nc.sync.dma_start(bc_t[:], bc_d[:])

            xs_tiles = []
            for mb in range(NMB):
                t = xs_pool.tile([P, CB], mmdt, tag="xs")
                nc.sync.dma_start(t[:], xs_d[mb * P:(mb + 1) * P, :])
                xs_tiles.append(t)
            xin_tiles = []
            for mb in range(NMB):
                t = xin_pool.tile([P, BC * IN], f32, tag="xin")
                nc.sync.dma_start(t[:], xin_d[mb * P:(mb + 1) * P, :])
                xin_tiles.append(t)

            ruT_tiles = [ruT_pool.tile([P, N], f32, tag="ruT", name=f"ruT{i}")
                         for i in range(BC)]

            def big_phase(x_tiles, w_tiles, out_rows, bias_t, act_fn,
                          out_slice_fn):
                """One graph-conv pass + projection + activation.

                out_slice_fn(i, k0, width) -> SBUF AP [out_rows, width]
                receiving act(proj + bias) for batch i, k cols [k0, k0+w).
                """
                with tc.tile_pool(name="stst", bufs=6) as st_pool, \
                     tc.tile_pool(name="agg", bufs=16) as agg_pool, \
                     tc.tile_pool(name="aggT", bufs=8) as aggT_pool, \
                     tc.tile_pool(name="aggps", bufs=4, space="PSUM") as agg_ps_pool, \
                     tc.tile_pool(name="tpps", bufs=4, space="PSUM") as tp_ps_pool:
                    for g in range(NG):
                        k0 = g * KBG * P        # 256-aligned k offset
                        agg_sb = {}
                        for j in range(J):
                            st_ts = []
                            for mq in range(NMB // MBQ):
                                st_t = st_pool.tile([P, MBQ, KBG * P], mmdt,
                                                    tag="st")
                                src = st_d[j, mq * MBQ * P:(mq + 1) * MBQ * P,
                                           k0:k0 + KBG * P]
                                src = src.rearrange("(g p) k -> p g k", p=P)
                                nc.sync.dma_start(st_t[:], src)
                                st_ts.append(st_t)
                            # kb/h-major, mb-minor: each psum tile's
                            # accumulation closes early so its drain overlaps
                            # the next tile's matmuls.
                            for kb in range(KBG):
                                t = agg_pool.tile([P, CB], f32, tag="agg",
                                                  name=f"agg{j}_{kb}")
                                for h in range(2):
                                    pst = agg_ps_pool.tile(
                                        [P, HALF], f32, tag="aggps",
                                        name=f"aggps{kb}_{h}")
                                    for mb in range(NMB):
                                        mq, ml = divmod(mb, MBQ)
                                        lhsT = st_ts[mq][:, ml,
                                                         kb * P:(kb + 1) * P]
                                        nc.tensor.matmul(
                                            pst[:],
                                            lhsT,
                                            x_tiles[mb][:, h * HALF:(h + 1) * HALF],
                                            start=(mb == 0),
                                            stop=(mb == NMB - 1),
                                        )
                                    if (kb + h) % 2 == 0:
                                        nc.vector.tensor_copy(
                                            t[:, h * HALF:(h + 1) * HALF],
                                            pst[:])
                                    else:
                                        nc.scalar.copy(
                                            t[:, h * HALF:(h + 1) * HALF],
                                            pst[:])
                                agg_sb[(j, kb)] = t

                        for i in range(BC):
                            aggT = []
                            for j in range(J):
                                tp = tp_ps_pool.tile([F, KBG * P], f32,
                                                     tag="tpproj",
                                                     name=f"tp{i}_{j}")
                                for kb in range(KBG):
                                    nc.tensor.transpose(
                                        tp[:, kb * P:(kb + 1) * P],
                                        agg_sb[(j, kb)][:, i * F:(i + 1) * F],
                                        ident[:])
                                at = aggT_pool.tile([F, KBG * P], f32r,
                                                    tag="aggT",
                                                    name=f"aggT{i}_{j}")
                                if (i + j) % 2 == 0:
                                    nc.vector.tensor_copy(at[:], tp[:])
                                else:
                                    nc.scalar.copy(at[:], tp[:])
                                aggT.append(at)
                            pp = tp_ps_pool.tile([out_rows, KBG * P], f32,
                                                 tag="tpproj",
                                                 name=f"proj{i}")
                            for j in range(J):
                                nc.tensor.matmul(
                                    pp[:],
                                    w_tiles[j][:],
                                    aggT[j][:],
                                    start=(j == 0),
                                    stop=(j == J - 1),
                                )
                            nc.scalar.activation(
                                out_slice_fn(i, k0, KBG * P), pp[:], act_fn,
                                bias=bias_t[:, 0:1])

            # ---- phase 1: r|u = sigmoid(graph_conv(xs, Wr|Wu)) ----
            big_phase(
                xs_tiles, wru_t, 2 * OUT, bru_t, AF.Sigmoid,
                lambda i, k0, w: ruT_tiles[i][:, k0:k0 + w])

            # ---- boundary: xc = [input, r*state] in [m, (i,f)] layout ----
            xc_tiles = [xs_pool.tile([P, CB], mmdt, tag="xs", name=f"xc{mb}")
                        for mb in range(NMB)]
            with tc.tile_pool(name="rstp", bufs=2, space="PSUM") as rstp_pool, \
                 tc.tile_pool(name="rsT", bufs=2) as rsT_pool:
                for mb in range(NMB):
                    dst = xc_tiles[mb][:].rearrange("p (i f) -> p i f", f=F)
                    src = xin_tiles[mb][:].rearrange("p (i f) -> p i f", f=IN)
                    nc.vector.tensor_copy(dst[:, :, 0:IN], src)
                for i in range(BC):
                    stt = stT_pool.tile([OUT, N], f32, tag="stT")
                    nc.sync.dma_start(stt[:], stT_d[i])
                    rst = rsT_pool.tile([OUT, N], f32, tag="rsT")
                    nc.vector.tensor_mul(rst[:], ruT_tiles[i][0:OUT, :],
                                         stt[:])
                    for mb in range(NMB):
                        tp = rstp_pool.tile([P, OUT], f32, tag="rstp")
                        nc.tensor.transpose(tp[:], rst[:, mb * P:(mb + 1) * P],
                                            ident[0:OUT, 0:OUT])
                        if mb % 2 == 0:
                            nc.vector.tensor_copy(
                                xc_tiles[mb][:, i * F + IN:(i + 1) * F], tp[:])
                        else:
                            nc.scalar.copy(
                                xc_tiles[mb][:, i * F + IN:(i + 1) * F], tp[:])

            # ---- phase 2: c.T = tanh(proj) overwrites dead r.T rows ----
            big_phase(
                xc_tiles, wc_t, OUT, bc_t, AF.Tanh,
                lambda i, k0, w: ruT_tiles[i][0:OUT, k0:k0 + w])

            # ---- GRU combine: out = c + u*(state - c) ----
            with tc.tile_pool(name="tmp", bufs=3) as tmp_pool:
                for i in range(BC):
                    stt = stT_pool.tile([OUT, N], f32, tag="stT")
                    nc.sync.dma_start(stt[:], stT_d[i])
                    u0 = tmp_pool.tile([OUT, N], f32, tag="tmp")
                    # partition-base shift (64 -> 0) needs a DMA, not DVE
                    nc.sync.dma_start(u0[:], ruT_tiles[i][OUT:2 * OUT, :])
                    t1 = tmp_pool.tile([OUT, N], f32, tag="tmp")
                    nc.vector.tensor_sub(t1[:], stt[:], ruT_tiles[i][0:OUT, :])
                    t2 = tmp_pool.tile([OUT, N], f32, tag="tmp")
                    nc.vector.tensor_mul(t2[:], u0[:], t1[:])
                    t3 = tmp_pool.tile([OUT, N], f32, tag="tmp")
                    nc.vector.tensor_add(t3[:], ruT_tiles[i][0:OUT, :], t2[:])
                    nc.sync.dma_start(outT_d[i], t3[:])

    nc.compile()
    return nc


def _get_module():
    if "nc" not in _CACHE:
        _CACHE["nc"] = _build_module()
    return _CACHE["nc"]


def kernel(input, state, supports, Wr, br, Wu, bu, Wc, bc):
    input = np.asarray(input, np.float32)
    state = np.asarray(state, np.float32)
    supports = np.asarray(supports, np.float32)
    Wr = np.asarray(Wr, np.float32)
    br = np.asarray(br, np.float32)
    Wu = np.asarray(Wu, np.float32)
    bu = np.asarray(bu, np.float32)
    Wc = np.asarray(Wc, np.float32)
    bc = np.asarray(bc, np.float32)

    from concourse.bass_utils import run_bass_kernel_spmd

    nc = _get_module()

    mmnp = np.float16 if MM16 else np.float32
    st_host = np.ascontiguousarray(supports.transpose(0, 2, 1).astype(mmnp))
    wru = np.ascontiguousarray(np.concatenate([Wr, Wu], axis=2))
    bru = np.concatenate([br, bu]).reshape(2 * OUT, 1).astype(np.float32)
    bcc = bc.reshape(OUT, 1).astype(np.float32)
    xs_full = np.concatenate([input, state], axis=2)  # [B, N, F]

    in_maps = []
    for c in range(NCORES):
        sl = slice(c * BC, (c + 1) * BC)
        xs_c = np.ascontiguousarray(
            xs_full[sl].transpose(1, 0, 2).reshape(N, CB).astype(mmnp))
        xin_c = np.ascontiguousarray(
            input[sl].transpose(1, 0, 2).reshape(N, BC * IN))
        stT_c = np.ascontiguousarray(state[sl].transpose(0, 2, 1))
        in_maps.append({
            "st": st_host,
            "xs": xs_c,
            "xin": xin_c,
            "stT": stT_c,
            "wru": wru,
            "wc": np.ascontiguousarray(Wc),
            "bru": bru,
            "bc": bcc,
        })

    import time
    t0 = time.monotonic()
    res = run_bass_kernel_spmd(nc, in_maps, core_ids=list(range(NCORES)))
    _CACHE["last_wall_s"] = time.monotonic() - t0

    out = np.empty((B, N, OUT), np.float32)
    for c in range(NCORES):
        outT = res.results[c]["outT"]           # [BC, OUT, N]
        out[c * BC:(c + 1) * BC] = outT.transpose(0, 2, 1)
    return out
